# revision 9
# baseline (speedup 1.0000x reference)
# Multi-head attention (B=2, S=4096, D=512, H=8) on 8 trn2 NeuronCores.
#
# Sharding: core c -> batch b=c//4, head-pair p=c%4 (heads 2p, 2p+1).
# Each core computes its two heads' attention plus the partial output
# projection restricted to those heads' columns of Wo; the host sums the
# 4 partials per batch and adds bo. No cross-device communication.
#
# Device-side layout is fully "transposed": Q^T/K^T/V^T [head_dim, S]
# come straight out of the projection matmuls (weights stationary, x^T
# streaming), scores are computed as S^T[k, q] so the PV matmul needs no
# transposes, and a ones-column appended to V makes the PV accumulation
# produce softmax denominators for free. exp() runs on the scalar engine
# reading score PSUM directly (scale=1/8 folded in); softmax max-
# subtraction is skipped (scores are O(1) here, exp cannot overflow).

import numpy as np

D_MODEL = 512
NUM_HEADS = 8
D_K = 64
B, S = 2, 4096
N_CORES = 8

_CACHE = {}


def _build_nc():
    from concourse import bacc, mybir
    import concourse.tile as tile
    from concourse.bass import ts

    f32 = mybir.dt.float32
    f32r = mybir.dt.float32r
    bf16 = mybir.dt.bfloat16
    Exp = mybir.ActivationFunctionType.Exp


    nc = bacc.Bacc("TRN2", target_bir_lowering=False, debug=False)

    xT_d = nc.dram_tensor("xT", [512, S], f32r, kind="ExternalInput")
    wq_d = nc.dram_tensor("wq2", [512, 128], f32r, kind="ExternalInput")
    wk_d = nc.dram_tensor("wk2", [512, 128], f32r, kind="ExternalInput")
    wv_d = nc.dram_tensor("wv2", [512, 128], f32r, kind="ExternalInput")
    bq_d = nc.dram_tensor("bq2", [128, 1], f32, kind="ExternalInput")
    bk_d = nc.dram_tensor("bk2", [128, 1], f32, kind="ExternalInput")
    bv_d = nc.dram_tensor("bv2", [128, 1], f32, kind="ExternalInput")
    wo_d = nc.dram_tensor("wo2", [128, 512], bf16, kind="ExternalInput")
    id_d = nc.dram_tensor("ident", [128, 64], f32, kind="ExternalInput")
    outT_d = nc.dram_tensor("outT", [512, S], f32, kind="ExternalOutput")

    NT = S // 512  # 8 q-tiles of 512
    NCK = S // 128  # 32 k-chunks of 128

    with tile.TileContext(nc) as tc:
        with (
            tc.tile_pool(name="const", bufs=1) as constp,
            tc.tile_pool(name="big", bufs=1) as bigp,
            tc.tile_pool(name="expool", bufs=2) as expool,
            tc.tile_pool(name="stage", bufs=3) as stagep,
            tc.tile_pool(name="scp", bufs=1, space="PSUM") as scp,
            tc.tile_pool(name="pvp", bufs=1, space="PSUM") as pvp,
        ):
            # ---- constants / inputs ----
            xT = bigp.tile([128, 4, S], f32r, tag="xT")
            nc.sync.dma_start(
                out=xT, in_=xT_d.ap().rearrange("(c p) s -> p c s", p=128)
            )
            wq = constp.tile([128, 4, 128], f32r, tag="wq")
            nc.sync.dma_start(
                out=wq, in_=wq_d.ap().rearrange("(c p) m -> p c m", p=128)
            )
            wk = constp.tile([128, 4, 128], f32r, tag="wk")
            nc.sync.dma_start(
                out=wk, in_=wk_d.ap().rearrange("(c p) m -> p c m", p=128)
            )
            wv = constp.tile([128, 4, 128], f32r, tag="wv")
            nc.sync.dma_start(
                out=wv, in_=wv_d.ap().rearrange("(c p) m -> p c m", p=128)
            )
            bq = constp.tile([128, 1], f32, tag="bq")
            nc.sync.dma_start(out=bq, in_=bq_d.ap())
            bk = constp.tile([128, 1], f32, tag="bk")
            nc.sync.dma_start(out=bk, in_=bk_d.ap())
            bv = constp.tile([128, 1], f32, tag="bv")
            nc.sync.dma_start(out=bv, in_=bv_d.ap())
            wo = constp.tile([128, 512], bf16, tag="wo")
            nc.sync.dma_start(out=wo, in_=wo_d.ap())
            ones64 = constp.tile([1, 64], f32, tag="ones64")
            nc.vector.memset(ones64, 1.0)
            ident = constp.tile([128, 64], f32, tag="ident")
            nc.sync.dma_start(out=ident, in_=id_d.ap())

            # ---- projections: QT2/KT2/VT2 [128, S] (head h in rows 64h..64h+63)
            QT2 = bigp.tile([128, S], bf16, tag="QT2")
            KT2 = bigp.tile([128, S], bf16, tag="KT2")
            VT2 = bigp.tile([128, S], f32, tag="VT2")
            for w_sb, b_sb, dst in ((wq, bq, QT2), (wk, bk, KT2), (wv, bv, VT2)):
                for t in range(NT):
                    pps = scp.tile([128, 1024], f32, tag=f"sc{t % 2}", name="pps")
                    for j in range(4):
                        nc.tensor.matmul(
                            pps[:, 0:512],
                            w_sb[:, j, :],
                            xT[:, j, ts(t, 512)],
                            start=(j == 0),
                            stop=(j == 3),
                        )
                    nc.vector.tensor_scalar_add(
                        out=dst[:, ts(t, 512)], in0=pps[:, 0:512], scalar1=b_sb
                    )

            # ---- V to natural layout [k, 65] per head, ones in column 0 ----
            V0 = bigp.tile([128, NCK, 65], bf16, tag="V0")
            V1 = bigp.tile([128, NCK, 65], bf16, tag="V1")
            nc.vector.memset(V0[:, :, 64:65], 1.0)
            nc.vector.memset(V1[:, :, 64:65], 1.0)
            for h, V in ((0, V0), (1, V1)):
                for ck in range(NCK):
                    vtp = pvp.tile([128, 64], f32, tag=f"pv{ck % 4}", name="vtp")
                    nc.tensor.transpose(
                        out=vtp,
                        in_=VT2[64 * h : 64 * h + 64, ts(ck, 128)],
                        identity=ident[64 * h : 64 * h + 64, :],
                    )
                    nc.vector.tensor_copy(out=V[:, ck, 0:64], in_=vtp)

            # ---- attention + output projection ----
            attnT2 = bigp.tile([128, S], bf16, tag="attnT2")
            for tp_i in range(NT // 2):
                pv = [
                    [
                        pvp.tile([65, 512], f32, tag=f"pv{2 * h + par}", name="pv")
                        for par in range(2)
                    ]
                    for h in range(2)
                ]
                for ck in range(NCK):
                    for par in range(2):
                        t = 2 * tp_i + par
                        sc = scp.tile([128, 1024], f32, tag=f"sc{par}", name="sc")
                        nc.tensor.matmul(
                            sc[:, 0:512],
                            KT2[0:64, ts(ck, 128)],
                            QT2[0:64, ts(t, 512)],
                            start=True,
                            stop=True,
                        )
                        nc.tensor.matmul(
                            sc[:, 512:1024],
                            KT2[64:128, ts(ck, 128)],
                            QT2[64:128, ts(t, 512)],
                            start=True,
                            stop=True,
                        )
                        ex = expool.tile([128, 1024], bf16, tag=f"ex{par}", name="ex")
                        nc.scalar.activation(out=ex, in_=sc, func=Exp, scale=0.125)
                        nc.tensor.matmul(
                            pv[0][par],
                            V0[:, ck, :],
                            ex[:, 0:512],
                            start=(ck == 0),
                            stop=(ck == NCK - 1),
                        )
                        nc.tensor.matmul(
                            pv[1][par],
                            V1[:, ck, :],
                            ex[:, 512:1024],
                            start=(ck == 0),
                            stop=(ck == NCK - 1),
                        )
                # normalize: attnT2[:, t] = pv[h][par][1:65] * (1/denom) rows
                for par in range(2):
                    t = 2 * tp_i + par
                    rc0 = stagep.tile([1, 512], f32, tag="rc0", name="rc0")
                    rc1 = stagep.tile([1, 512], f32, tag="rc1", name="rc1")
                    nc.vector.reciprocal(out=rc0, in_=pv[0][par][64:65, :])
                    nc.vector.reciprocal(out=rc1, in_=pv[1][par][64:65, :])
                    bc = scp.tile([128, 1024], f32, tag=f"sc{par}", name="bc")
                    nc.tensor.matmul(
                        bc[0:64, 0:512], ones64, rc0, start=True, stop=True
                    )
                    nc.tensor.matmul(
                        bc[64:128, 0:512],
                        ones64,
                        rc1,
                        start=True,
                        stop=True,
                        tile_position=(0, 64),
                    )
                    nc.vector.tensor_copy(
                        out=attnT2[0:64, ts(t, 512)], in_=pv[0][par][0:64, :]
                    )
                    nc.vector.tensor_copy(
                        out=attnT2[64:128, ts(t, 512)], in_=pv[1][par][0:64, :]
                    )
                    nc.vector.tensor_mul(
                        attnT2[:, ts(t, 512)],
                        attnT2[:, ts(t, 512)],
                        bc[:, 0:512],
                    )
                # output projection for the two finished q-tiles
                for m in range(4):
                    for par in range(2):
                        t = 2 * tp_i + par
                        ops = scp.tile([128, 1024], f32, tag=f"sc{par}", name="ops")
                        nc.tensor.matmul(
                            ops[:, 0:512],
                            wo[:, ts(m, 128)],
                            attnT2[:, ts(t, 512)],
                            start=True,
                            stop=True,
                        )
                        ost = stagep.tile([128, 512], f32, tag="ostage", name="ost")
                        nc.vector.tensor_copy(out=ost, in_=ops[:, 0:512])
                        nc.sync.dma_start(
                            out=outT_d.ap()[ts(m, 128), ts(t, 512)], in_=ost
                        )

    nc.compile()
    return nc


def _get_nc():
    if "nc" not in _CACHE:
        _CACHE["nc"] = _build_nc()
    return _CACHE["nc"]


def _bf16np():
    import ml_dtypes

    return ml_dtypes.bfloat16


def _make_in_maps(inputs):
    x = np.ascontiguousarray(np.asarray(inputs["x"], dtype=np.float32))
    Wq = np.asarray(inputs["Wq"], dtype=np.float32)
    Wk = np.asarray(inputs["Wk"], dtype=np.float32)
    Wv = np.asarray(inputs["Wv"], dtype=np.float32)
    Wo = np.asarray(inputs["Wo"], dtype=np.float32)
    bq = np.asarray(inputs["bq"], dtype=np.float32)
    bk = np.asarray(inputs["bk"], dtype=np.float32)
    bv = np.asarray(inputs["bv"], dtype=np.float32)

    ident = np.concatenate([np.eye(64, dtype=np.float32)] * 2, axis=0)

    in_maps = []
    for c in range(N_CORES):
        b, p = c // 4, c % 4
        hs = slice(128 * p, 128 * (p + 1))
        in_maps.append(
            {
                "xT": np.ascontiguousarray(x[b].T),
                "wq2": np.ascontiguousarray(Wq[hs, :].T),
                "wk2": np.ascontiguousarray(Wk[hs, :].T),
                "wv2": np.ascontiguousarray(Wv[hs, :].T),
                "bq2": np.ascontiguousarray(bq[hs]).reshape(128, 1),
                "bk2": np.ascontiguousarray(bk[hs]).reshape(128, 1),
                "bv2": np.ascontiguousarray(bv[hs]).reshape(128, 1),
                "wo2": np.ascontiguousarray(Wo[:, hs].T).astype(_bf16np()),
                "ident": ident,
            }
        )
    return in_maps


def _gather(results, inputs):
    bo = np.asarray(inputs["bo"], dtype=np.float32)
    out = np.zeros((B, S, D_MODEL), np.float32)
    for c in range(N_CORES):
        out[c // 4] += results[c]["outT"].T
    out += bo[None, None, :]
    return out


def kernel(**inputs):
    from concourse.bass_utils import run_bass_kernel_spmd

    nc = _get_nc()
    in_maps = _make_in_maps(inputs)
    res = run_bass_kernel_spmd(nc, in_maps, list(range(N_CORES)))
    return _gather(res.results, inputs)


# revision 10
# speedup vs baseline: 1.3465x; 1.3465x over previous
# Multi-head attention (B=2, S=4096, D=512, H=8) on 8 trn2 NeuronCores.
#
# Sharding: core c -> batch b=c//4, head-pair p=c%4 (heads 2p, 2p+1).
# Each core computes its two heads' attention plus the partial output
# projection restricted to those heads' columns of Wo; the host sums the
# 4 partials per batch and adds bo. No cross-device communication.
#
# Device-side layout is fully "transposed": Q^T/K^T/V^T [head_dim, S]
# come straight out of the projection matmuls (weights stationary, x^T
# streaming), scores are computed as S^T[k, q] so the PV matmul needs no
# transposes, and a ones-column appended to V makes the PV accumulation
# produce softmax denominators for free. exp() runs on the scalar engine
# reading score PSUM directly (scale=1/8 folded in); softmax max-
# subtraction is skipped (scores are O(1) here, exp cannot overflow).

import numpy as np

D_MODEL = 512
NUM_HEADS = 8
D_K = 64
B, S = 2, 4096
N_CORES = 8

_CACHE = {}


def _build_nc():
    from concourse import bacc, mybir
    import concourse.tile as tile
    from concourse.bass import ts

    f32 = mybir.dt.float32
    f32r = mybir.dt.float32r
    bf16 = mybir.dt.bfloat16
    Exp = mybir.ActivationFunctionType.Exp


    nc = bacc.Bacc("TRN2", target_bir_lowering=False, debug=False)

    xT_d = nc.dram_tensor("xT", [512, S], f32r, kind="ExternalInput")
    wq_d = nc.dram_tensor("wq2", [512, 128], f32r, kind="ExternalInput")
    wk_d = nc.dram_tensor("wk2", [512, 128], f32r, kind="ExternalInput")
    wv_d = nc.dram_tensor("wv2", [512, 128], f32r, kind="ExternalInput")
    bq_d = nc.dram_tensor("bq2", [128, 1], f32, kind="ExternalInput")
    bk_d = nc.dram_tensor("bk2", [128, 1], f32, kind="ExternalInput")
    bv_d = nc.dram_tensor("bv2", [128, 1], f32, kind="ExternalInput")
    wo_d = nc.dram_tensor("wo2", [128, 512], bf16, kind="ExternalInput")
    id_d = nc.dram_tensor("ident", [128, 64], f32, kind="ExternalInput")
    outT_d = nc.dram_tensor("outT", [512, S], f32, kind="ExternalOutput")

    NT = S // 512  # 8 q-tiles of 512
    NCK = S // 128  # 32 k-chunks of 128

    with tile.TileContext(nc) as tc:
        with (
            tc.tile_pool(name="const", bufs=1) as constp,
            tc.tile_pool(name="big", bufs=1) as bigp,
            tc.tile_pool(name="expool", bufs=2) as expool,
            tc.tile_pool(name="stage", bufs=3) as stagep,
            tc.tile_pool(name="scp", bufs=1, space="PSUM") as scp,
            tc.tile_pool(name="pvp", bufs=1, space="PSUM") as pvp,
        ):
            # ---- constants / inputs ----
            xT = bigp.tile([128, 4, S], f32r, tag="xT")
            nc.sync.dma_start(
                out=xT, in_=xT_d.ap().rearrange("(c p) s -> p c s", p=128)
            )
            wq = constp.tile([128, 4, 128], f32r, tag="wq")
            nc.sync.dma_start(
                out=wq, in_=wq_d.ap().rearrange("(c p) m -> p c m", p=128)
            )
            wk = constp.tile([128, 4, 128], f32r, tag="wk")
            nc.sync.dma_start(
                out=wk, in_=wk_d.ap().rearrange("(c p) m -> p c m", p=128)
            )
            wv = constp.tile([128, 4, 128], f32r, tag="wv")
            nc.sync.dma_start(
                out=wv, in_=wv_d.ap().rearrange("(c p) m -> p c m", p=128)
            )
            bq = constp.tile([128, 1], f32, tag="bq")
            nc.sync.dma_start(out=bq, in_=bq_d.ap())
            bk = constp.tile([128, 1], f32, tag="bk")
            nc.sync.dma_start(out=bk, in_=bk_d.ap())
            bv = constp.tile([128, 1], f32, tag="bv")
            nc.sync.dma_start(out=bv, in_=bv_d.ap())
            wo = constp.tile([128, 512], bf16, tag="wo")
            nc.sync.dma_start(out=wo, in_=wo_d.ap())
            ones64 = constp.tile([1, 64], f32, tag="ones64")
            nc.vector.memset(ones64, 1.0)
            ident = constp.tile([128, 64], f32, tag="ident")
            nc.sync.dma_start(out=ident, in_=id_d.ap())

            # ---- projections: QT2/KT2/VT2 [128, S] (head h in rows 64h..64h+63)
            QT2 = bigp.tile([128, S], bf16, tag="QT2")
            KT2 = bigp.tile([128, S], bf16, tag="KT2")
            VT2 = bigp.tile([128, S], f32, tag="VT2")
            for w_sb, b_sb, dst in ((wq, bq, QT2), (wk, bk, KT2), (wv, bv, VT2)):
                for t in range(NT):
                    pps = scp.tile([128, 1024], f32, tag=f"sc{t % 2}", name="pps")
                    for j in range(4):
                        nc.tensor.matmul(
                            pps[:, 0:512],
                            w_sb[:, j, :],
                            xT[:, j, ts(t, 512)],
                            start=(j == 0),
                            stop=(j == 3),
                        )
                    nc.vector.tensor_scalar_add(
                        out=dst[:, ts(t, 512)], in0=pps[:, 0:512], scalar1=b_sb
                    )

            # ---- V to natural layout [k, 65] per head, ones in column 0 ----
            V0 = bigp.tile([128, NCK, 65], bf16, tag="V0")
            V1 = bigp.tile([128, NCK, 65], bf16, tag="V1")
            nc.vector.memset(V0[:, :, 64:65], 1.0)
            nc.vector.memset(V1[:, :, 64:65], 1.0)
            for h, V in ((0, V0), (1, V1)):
                for ck in range(NCK):
                    vtp = pvp.tile([128, 64], f32, tag=f"pv{ck % 4}", name="vtp")
                    nc.tensor.transpose(
                        out=vtp,
                        in_=VT2[64 * h : 64 * h + 64, ts(ck, 128)],
                        identity=ident[64 * h : 64 * h + 64, :],
                    )
                    nc.vector.tensor_copy(out=V[:, ck, 0:64], in_=vtp)

            # ---- attention + output projection ----
            attnT2 = bigp.tile([128, S], bf16, tag="attnT2")
            for tp_i in range(NT // 2):
                pv = [
                    [
                        pvp.tile([65, 512], f32, tag=f"pv{2 * h + par}", name="pv")
                        for par in range(2)
                    ]
                    for h in range(2)
                ]
                # software pipeline: pv matmuls for k-chunk ck are emitted
                # during iteration ck+1, so the in-order PE never waits on the
                # exp() it just triggered.
                def emit_pv(ck, exs):
                    for par in range(2):
                        nc.tensor.matmul(
                            pv[0][par],
                            V0[:, ck, :],
                            exs[par][:, 0:512],
                            start=(ck == 0),
                            stop=(ck == NCK - 1),
                        )
                        nc.tensor.matmul(
                            pv[1][par],
                            V1[:, ck, :],
                            exs[par][:, 512:1024],
                            start=(ck == 0),
                            stop=(ck == NCK - 1),
                        )

                prev = None
                for ck in range(NCK):
                    exs = []
                    for par in range(2):
                        t = 2 * tp_i + par
                        sc = scp.tile([128, 1024], f32, tag=f"sc{par}", name="sc")
                        nc.tensor.matmul(
                            sc[:, 0:512],
                            KT2[0:64, ts(ck, 128)],
                            QT2[0:64, ts(t, 512)],
                            start=True,
                            stop=True,
                        )
                        nc.tensor.matmul(
                            sc[:, 512:1024],
                            KT2[64:128, ts(ck, 128)],
                            QT2[64:128, ts(t, 512)],
                            start=True,
                            stop=True,
                        )
                        ex = expool.tile([128, 1024], bf16, tag=f"ex{par}", name="ex")
                        nc.scalar.activation(out=ex, in_=sc, func=Exp, scale=0.125)
                        exs.append(ex)
                    if prev is not None:
                        emit_pv(prev[0], prev[1])
                    prev = (ck, exs)
                emit_pv(prev[0], prev[1])
                # normalize: attnT2[:, t] = pv[h][par][1:65] * (1/denom) rows
                for par in range(2):
                    t = 2 * tp_i + par
                    rc0 = stagep.tile([1, 512], f32, tag="rc0", name="rc0")
                    rc1 = stagep.tile([1, 512], f32, tag="rc1", name="rc1")
                    den0 = stagep.tile([1, 512], f32, tag="den0", name="den0")
                    den1 = stagep.tile([1, 512], f32, tag="den1", name="den1")
                    nc.vector.tensor_copy(out=den0, in_=pv[0][par][64:65, :])
                    nc.vector.tensor_copy(out=den1, in_=pv[1][par][64:65, :])
                    nc.vector.reciprocal_approx_fast(out=rc0, in_=den0)
                    nc.vector.reciprocal_approx_fast(out=rc1, in_=den1)
                    bc = scp.tile([128, 1024], f32, tag=f"sc{par}", name="bc")
                    nc.tensor.matmul(
                        bc[0:64, 0:512], ones64, rc0, start=True, stop=True
                    )
                    nc.tensor.matmul(
                        bc[64:128, 0:512],
                        ones64,
                        rc1,
                        start=True,
                        stop=True,
                        tile_position=(0, 64),
                    )
                    nc.vector.tensor_copy(
                        out=attnT2[0:64, ts(t, 512)], in_=pv[0][par][0:64, :]
                    )
                    nc.vector.tensor_copy(
                        out=attnT2[64:128, ts(t, 512)], in_=pv[1][par][0:64, :]
                    )
                    nc.vector.tensor_mul(
                        attnT2[:, ts(t, 512)],
                        attnT2[:, ts(t, 512)],
                        bc[:, 0:512],
                    )
                # output projection for the two finished q-tiles
                for m in range(4):
                    for par in range(2):
                        t = 2 * tp_i + par
                        ops = scp.tile([128, 1024], f32, tag=f"sc{par}", name="ops")
                        nc.tensor.matmul(
                            ops[:, 0:512],
                            wo[:, ts(m, 128)],
                            attnT2[:, ts(t, 512)],
                            start=True,
                            stop=True,
                        )
                        ost = stagep.tile([128, 512], f32, tag="ostage", name="ost")
                        nc.vector.tensor_copy(out=ost, in_=ops[:, 0:512])
                        nc.sync.dma_start(
                            out=outT_d.ap()[ts(m, 128), ts(t, 512)], in_=ost
                        )

    nc.compile()
    return nc


def _get_nc():
    if "nc" not in _CACHE:
        _CACHE["nc"] = _build_nc()
    return _CACHE["nc"]


def _bf16np():
    import ml_dtypes

    return ml_dtypes.bfloat16


def _make_in_maps(inputs):
    x = np.ascontiguousarray(np.asarray(inputs["x"], dtype=np.float32))
    Wq = np.asarray(inputs["Wq"], dtype=np.float32)
    Wk = np.asarray(inputs["Wk"], dtype=np.float32)
    Wv = np.asarray(inputs["Wv"], dtype=np.float32)
    Wo = np.asarray(inputs["Wo"], dtype=np.float32)
    bq = np.asarray(inputs["bq"], dtype=np.float32)
    bk = np.asarray(inputs["bk"], dtype=np.float32)
    bv = np.asarray(inputs["bv"], dtype=np.float32)

    ident = np.concatenate([np.eye(64, dtype=np.float32)] * 2, axis=0)

    in_maps = []
    for c in range(N_CORES):
        b, p = c // 4, c % 4
        hs = slice(128 * p, 128 * (p + 1))
        in_maps.append(
            {
                "xT": np.ascontiguousarray(x[b].T),
                "wq2": np.ascontiguousarray(Wq[hs, :].T),
                "wk2": np.ascontiguousarray(Wk[hs, :].T),
                "wv2": np.ascontiguousarray(Wv[hs, :].T),
                "bq2": np.ascontiguousarray(bq[hs]).reshape(128, 1),
                "bk2": np.ascontiguousarray(bk[hs]).reshape(128, 1),
                "bv2": np.ascontiguousarray(bv[hs]).reshape(128, 1),
                "wo2": np.ascontiguousarray(Wo[:, hs].T).astype(_bf16np()),
                "ident": ident,
            }
        )
    return in_maps


def _gather(results, inputs):
    bo = np.asarray(inputs["bo"], dtype=np.float32)
    out = np.zeros((B, S, D_MODEL), np.float32)
    for c in range(N_CORES):
        out[c // 4] += results[c]["outT"].T
    out += bo[None, None, :]
    return out


def kernel(**inputs):
    from concourse.bass_utils import run_bass_kernel_spmd

    nc = _get_nc()
    in_maps = _make_in_maps(inputs)
    res = run_bass_kernel_spmd(nc, in_maps, list(range(N_CORES)))
    return _gather(res.results, inputs)


# revision 13
# speedup vs baseline: 1.4867x; 1.1041x over previous
# Multi-head attention (B=2, S=4096, D=512, H=8) on 8 trn2 NeuronCores.
#
# Sharding: core c -> batch b=c//4, head-pair p=c%4 (heads 2p, 2p+1).
# Each core computes its two heads' attention plus the partial output
# projection restricted to those heads' columns of Wo; the host sums the
# 4 partials per batch and adds bo. No cross-device communication.
#
# Device-side layout is fully "transposed": Q^T/K^T [head_dim, S] come
# straight out of the projection matmuls (weights stationary, x^T
# streaming), scores are computed as S^T[k, q] so the PV matmul needs no
# transposes, and a ones-column appended to V makes the PV accumulation
# produce softmax denominators for free. exp() runs on the scalar engine
# reading score PSUM directly (scale=1/8 folded in); softmax max-
# subtraction is skipped (scores are O(1) here, exp cannot overflow).
# Attention matmuls run in bf16 (softmax normalization + long averaging
# damps the rounding noise); x^T is shipped as bf16 which also halves
# the input DMA. The scalar engine (exp) is the bottleneck; the PV
# matmuls are software-pipelined one k-chunk behind the scores so the
# in-order PE never waits on the exp it just triggered, and the
# normalization/output-projection epilogues are kept off the scalar
# engine's critical path.

import numpy as np

D_MODEL = 512
NUM_HEADS = 8
D_K = 64
B, S = 2, 4096
N_CORES = 8

_CACHE = {}


def _build_nc():
    from concourse import bacc, mybir
    import concourse.tile as tile
    from concourse.bass import ts

    f32 = mybir.dt.float32
    bf16 = mybir.dt.bfloat16
    Exp = mybir.ActivationFunctionType.Exp

    nc = bacc.Bacc("TRN2", target_bir_lowering=False, debug=False)

    xT_d = nc.dram_tensor("xT", [512, S], bf16, kind="ExternalInput")
    wq_d = nc.dram_tensor("wq2", [512, 128], bf16, kind="ExternalInput")
    wk_d = nc.dram_tensor("wk2", [512, 128], bf16, kind="ExternalInput")
    wv_d = nc.dram_tensor("wv2", [512, 128], bf16, kind="ExternalInput")
    bq_d = nc.dram_tensor("bq2", [128, 1], f32, kind="ExternalInput")
    bk_d = nc.dram_tensor("bk2", [128, 1], f32, kind="ExternalInput")
    bv_d = nc.dram_tensor("bv2", [128, 1], f32, kind="ExternalInput")
    wo_d = nc.dram_tensor("wo2", [128, 512], bf16, kind="ExternalInput")
    id_d = nc.dram_tensor("ident", [128, 64], f32, kind="ExternalInput")
    outT_d = nc.dram_tensor("outT", [512, S], f32, kind="ExternalOutput")

    NT = S // 512  # 8 q-tiles of 512
    NCK = S // 128  # 32 k-chunks of 128

    with tile.TileContext(nc) as tc:
        with (
            tc.tile_pool(name="const", bufs=1) as constp,
            tc.tile_pool(name="big", bufs=1) as bigp,
            tc.tile_pool(name="expool", bufs=2) as expool,
            tc.tile_pool(name="stage", bufs=2) as stagep,
            tc.tile_pool(name="ost", bufs=3) as ostp,
            tc.tile_pool(name="scp", bufs=1, space="PSUM") as scp,
            tc.tile_pool(name="pvp", bufs=1, space="PSUM") as pvp,
        ):
            # ---- constants ----
            wq = constp.tile([128, 4, 128], bf16, tag="wq")
            nc.sync.dma_start(
                out=wq, in_=wq_d.ap().rearrange("(c p) m -> p c m", p=128)
            )
            wk = constp.tile([128, 4, 128], bf16, tag="wk")
            nc.sync.dma_start(
                out=wk, in_=wk_d.ap().rearrange("(c p) m -> p c m", p=128)
            )
            wv = constp.tile([128, 4, 128], bf16, tag="wv")
            nc.sync.dma_start(
                out=wv, in_=wv_d.ap().rearrange("(c p) m -> p c m", p=128)
            )
            bq = constp.tile([128, 1], f32, tag="bq")
            nc.sync.dma_start(out=bq, in_=bq_d.ap())
            bk = constp.tile([128, 1], f32, tag="bk")
            nc.sync.dma_start(out=bk, in_=bk_d.ap())
            bv = constp.tile([128, 1], f32, tag="bv")
            nc.sync.dma_start(out=bv, in_=bv_d.ap())
            wo = constp.tile([128, 512], bf16, tag="wo")
            nc.sync.dma_start(out=wo, in_=wo_d.ap())
            ident = constp.tile([128, 64], f32, tag="ident")
            nc.sync.dma_start(out=ident, in_=id_d.ap())

            # ---- x^T load, split so projections can start early ----
            xT = bigp.tile([128, 4, S], bf16, tag="xT")
            xT_src = xT_d.ap().rearrange("(c p) s -> p c s", p=128)
            for j in range(4):
                for h in range(2):
                    nc.sync.dma_start(
                        out=xT[:, j, ts(h, 2048)], in_=xT_src[:, j, ts(h, 2048)]
                    )

            # ---- projections, one tile per 512-wide q/k slice ----
            # Emission order K(t) -> Q(t) -> V(t) so the attention stream
            # (which needs K t0 + Q t0/t1 first) starts as early as possible.
            QT2 = [
                bigp.tile([128, 512], bf16, tag=f"QT2_{t}", name="qt")
                for t in range(NT)
            ]
            KT2 = [
                bigp.tile([128, 512], bf16, tag=f"KT2_{t}", name="kt")
                for t in range(NT)
            ]
            VT2 = [
                bigp.tile([128, 512], f32, tag=f"VT2_{t}", name="vt")
                for t in range(NT)
            ]
            # V in natural layout per 128-k-chunk, ones in column 64
            V0 = [
                bigp.tile([128, 65], bf16, tag=f"V0_{ck}", name="v0")
                for ck in range(NCK)
            ]
            V1 = [
                bigp.tile([128, 65], bf16, tag=f"V1_{ck}", name="v1")
                for ck in range(NCK)
            ]

            def proj(t, w_sb, b_sb, dst, psum_tag):
                pps = scp.tile([128, 1024], f32, tag=psum_tag, name="pps")
                for j in range(4):
                    nc.tensor.matmul(
                        pps[:, 0:512],
                        w_sb[:, j, :],
                        xT[:, j, ts(t, 512)],
                        start=(j == 0),
                        stop=(j == 3),
                    )
                nc.vector.tensor_scalar_add(out=dst, in0=pps[:, 0:512], scalar1=b_sb)

            for t in range(NT):
                proj(t, wk, bk, KT2[t], f"sc{t % 2}")
                proj(t, wq, bq, QT2[t], f"sc{(t + 1) % 2}")
                proj(t, wv, bv, VT2[t], f"sc{t % 2}")
                for i in range(4):
                    ck = 4 * t + i
                    for h, V in ((0, V0), (1, V1)):
                        vtp = pvp.tile(
                            [128, 64], f32, tag=f"pv{(2 * i + h) % 4}", name="vtp"
                        )
                        nc.tensor.transpose(
                            out=vtp,
                            in_=VT2[t][64 * h : 64 * h + 64, ts(i, 128)],
                            identity=ident[64 * h : 64 * h + 64, :],
                        )
                        nc.vector.tensor_copy(out=V[ck][:, 0:64], in_=vtp)
                        nc.vector.memset(V[ck][:, 64:65], 1.0)

            # ---- attention ----
            attnT = [
                bigp.tile([128, 512], bf16, tag=f"attnT_{t}", name="at")
                for t in range(NT)
            ]

            def part_a(tp_i, pv):
                # pv-slot readers only: must be emitted before the next
                # tpair's first pv matmul reuses the slots.
                outs = []
                for par in range(2):
                    t = 2 * tp_i + par
                    den0 = stagep.tile([1, 512], f32, tag="den0", name="den0")
                    den1 = stagep.tile([1, 512], f32, tag="den1", name="den1")
                    nc.vector.tensor_copy(out=den0, in_=pv[0][par][64:65, :])
                    nc.vector.tensor_copy(out=den1, in_=pv[1][par][64:65, :])
                    nc.vector.tensor_copy(
                        out=attnT[t][0:64, :], in_=pv[0][par][0:64, :]
                    )
                    nc.vector.tensor_copy(
                        out=attnT[t][64:128, :], in_=pv[1][par][0:64, :]
                    )
                    outs.append((t, den0, den1))
                return outs

            def part_b(dens):
                # off the critical path: reciprocal + broadcast + normalize
                for t, den0, den1 in dens:
                    rc0 = stagep.tile([1, 512], f32, tag="rc0", name="rc0")
                    rc1 = stagep.tile([1, 512], f32, tag="rc1", name="rc1")
                    nc.vector.reciprocal_approx_fast(out=rc0, in_=den0)
                    nc.vector.reciprocal_approx_fast(out=rc1, in_=den1)
                    bct0 = stagep.tile([128, 512], f32, tag="bct0", name="bct0")
                    bct1 = stagep.tile([128, 512], f32, tag="bct1", name="bct1")
                    nc.gpsimd.partition_broadcast(bct0, rc0)
                    nc.gpsimd.partition_broadcast(bct1, rc1)
                    nc.vector.tensor_mul(
                        attnT[t][0:64, :], attnT[t][0:64, :], bct0[0:64, :]
                    )
                    nc.vector.tensor_mul(
                        attnT[t][64:128, :], attnT[t][64:128, :], bct1[64:128, :]
                    )

            pending = None
            for tp_i in range(NT // 2):
                pv = [
                    [
                        pvp.tile([65, 512], f32, tag=f"pv{2 * h + par}", name="pv")
                        for par in range(2)
                    ]
                    for h in range(2)
                ]

                def emit_pv(ck, exs, pv=pv):
                    for par in range(2):
                        nc.tensor.matmul(
                            pv[0][par],
                            V0[ck],
                            exs[par][:, 0:512],
                            start=(ck == 0),
                            stop=(ck == NCK - 1),
                        )
                        nc.tensor.matmul(
                            pv[1][par],
                            V1[ck],
                            exs[par][:, 512:1024],
                            start=(ck == 0),
                            stop=(ck == NCK - 1),
                        )

                prev = None
                for ck in range(NCK):
                    exs = []
                    for par in range(2):
                        t = 2 * tp_i + par
                        sc = scp.tile([128, 1024], f32, tag=f"sc{par}", name="sc")
                        nc.tensor.matmul(
                            sc[:, 0:512],
                            KT2[ck // 4][0:64, ts(ck % 4, 128)],
                            QT2[t][0:64, :],
                            start=True,
                            stop=True,
                        )
                        nc.tensor.matmul(
                            sc[:, 512:1024],
                            KT2[ck // 4][64:128, ts(ck % 4, 128)],
                            QT2[t][64:128, :],
                            start=True,
                            stop=True,
                        )
                        ex = expool.tile([128, 1024], bf16, tag=f"ex{par}", name="ex")
                        nc.scalar.activation(out=ex, in_=sc, func=Exp, scale=0.125)
                        exs.append(ex)
                    if prev is not None:
                        emit_pv(prev[0], prev[1])
                    prev = (ck, exs)
                    if ck == 2 and pending is not None:
                        # previous tpair's normalization, now well clear of
                        # the scalar-engine stream restart
                        part_b(pending)
                        pending = None
                emit_pv(prev[0], prev[1])
                pending = part_a(tp_i, pv)
            part_b(pending)

            # ---- output projection (all q-tiles at the end) ----
            for m in range(4):
                for t in range(NT):
                    ops = scp.tile([128, 1024], f32, tag=f"sc{t % 2}", name="ops")
                    nc.tensor.matmul(
                        ops[:, 0:512],
                        wo[:, ts(m, 128)],
                        attnT[t],
                        start=True,
                        stop=True,
                    )
                    ost = ostp.tile([128, 512], f32, tag="ostage", name="ost")
                    nc.vector.tensor_copy(out=ost, in_=ops[:, 0:512])
                    nc.sync.dma_start(
                        out=outT_d.ap()[ts(m, 128), ts(t, 512)], in_=ost
                    )

    nc.compile()
    return nc


def _get_nc():
    if "nc" not in _CACHE:
        _CACHE["nc"] = _build_nc()
    return _CACHE["nc"]


def _bf16np():
    import ml_dtypes

    return ml_dtypes.bfloat16


def _make_in_maps(inputs):
    x = np.ascontiguousarray(np.asarray(inputs["x"], dtype=np.float32))
    Wq = np.asarray(inputs["Wq"], dtype=np.float32)
    Wk = np.asarray(inputs["Wk"], dtype=np.float32)
    Wv = np.asarray(inputs["Wv"], dtype=np.float32)
    Wo = np.asarray(inputs["Wo"], dtype=np.float32)
    bq = np.asarray(inputs["bq"], dtype=np.float32)
    bk = np.asarray(inputs["bk"], dtype=np.float32)
    bv = np.asarray(inputs["bv"], dtype=np.float32)

    bf = _bf16np()
    ident = np.concatenate([np.eye(64, dtype=np.float32)] * 2, axis=0)

    in_maps = []
    for c in range(N_CORES):
        b, p = c // 4, c % 4
        hs = slice(128 * p, 128 * (p + 1))
        in_maps.append(
            {
                "xT": np.ascontiguousarray(x[b].T).astype(bf),
                "wq2": np.ascontiguousarray(Wq[hs, :].T).astype(bf),
                "wk2": np.ascontiguousarray(Wk[hs, :].T).astype(bf),
                "wv2": np.ascontiguousarray(Wv[hs, :].T).astype(bf),
                "bq2": np.ascontiguousarray(bq[hs]).reshape(128, 1),
                "bk2": np.ascontiguousarray(bk[hs]).reshape(128, 1),
                "bv2": np.ascontiguousarray(bv[hs]).reshape(128, 1),
                "wo2": np.ascontiguousarray(Wo[:, hs].T).astype(bf),
                "ident": ident,
            }
        )
    return in_maps


def _gather(results, inputs):
    bo = np.asarray(inputs["bo"], dtype=np.float32)
    out = np.zeros((B, S, D_MODEL), np.float32)
    for c in range(N_CORES):
        out[c // 4] += results[c]["outT"].T
    out += bo[None, None, :]
    return out


def kernel(**inputs):
    from concourse.bass_utils import run_bass_kernel_spmd

    nc = _get_nc()
    in_maps = _make_in_maps(inputs)
    res = run_bass_kernel_spmd(nc, in_maps, list(range(N_CORES)))
    return _gather(res.results, inputs)


# revision 14
# speedup vs baseline: 1.5495x; 1.0423x over previous
# Multi-head attention (B=2, S=4096, D=512, H=8) on 8 trn2 NeuronCores.
#
# Sharding: core c -> batch b=c//4, head-pair p=c%4 (heads 2p, 2p+1).
# Each core computes its two heads' attention plus the partial output
# projection restricted to those heads' columns of Wo; the host sums the
# 4 partials per batch and adds bo. No cross-device communication.
#
# Device-side layout is fully "transposed": Q^T/K^T [head_dim, S] come
# straight out of the projection matmuls (weights stationary, x^T
# streaming), scores are computed as S^T[k, q] so the PV matmul needs no
# transposes, and a ones-column appended to V makes the PV accumulation
# produce softmax denominators for free. exp() runs on the scalar engine
# reading score PSUM directly (scale=1/8 folded in); softmax max-
# subtraction is skipped (scores are O(1) here, exp cannot overflow).
# Attention matmuls run in bf16 (softmax normalization + long averaging
# damps the rounding noise); x^T is shipped as bf16 which also halves
# the input DMA. The scalar engine (exp) is the bottleneck; the PV
# matmuls are software-pipelined one k-chunk behind the scores so the
# in-order PE never waits on the exp it just triggered, and the
# normalization/output-projection epilogues are kept off the scalar
# engine's critical path.

import numpy as np

D_MODEL = 512
NUM_HEADS = 8
D_K = 64
B, S = 2, 4096
N_CORES = 8

_CACHE = {}


def _build_nc():
    from concourse import bacc, mybir
    import concourse.tile as tile
    from concourse.bass import ts

    f32 = mybir.dt.float32
    bf16 = mybir.dt.bfloat16
    Exp = mybir.ActivationFunctionType.Exp

    nc = bacc.Bacc("TRN2", target_bir_lowering=False, debug=False)

    xT_d = nc.dram_tensor("xT", [512, S], bf16, kind="ExternalInput")
    wq_d = nc.dram_tensor("wq2", [512, 128], bf16, kind="ExternalInput")
    wk_d = nc.dram_tensor("wk2", [512, 128], bf16, kind="ExternalInput")
    wv_d = nc.dram_tensor("wv2", [512, 128], bf16, kind="ExternalInput")
    bq_d = nc.dram_tensor("bq2", [128, 1], f32, kind="ExternalInput")
    bk_d = nc.dram_tensor("bk2", [128, 1], f32, kind="ExternalInput")
    bv_d = nc.dram_tensor("bv2", [128, 1], f32, kind="ExternalInput")
    wo_d = nc.dram_tensor("wo2", [128, 512], bf16, kind="ExternalInput")
    id_d = nc.dram_tensor("ident", [128, 64], f32, kind="ExternalInput")
    outT_d = nc.dram_tensor("outT", [512, S], f32, kind="ExternalOutput")

    NT = S // 512  # 8 q-tiles of 512
    NCK = S // 128  # 32 k-chunks of 128

    with tile.TileContext(nc) as tc:
        with (
            tc.tile_pool(name="const", bufs=1) as constp,
            tc.tile_pool(name="big", bufs=1) as bigp,
            tc.tile_pool(name="expool", bufs=2) as expool,
            tc.tile_pool(name="stage", bufs=2) as stagep,
            tc.tile_pool(name="ost", bufs=3) as ostp,
            tc.tile_pool(name="scp", bufs=1, space="PSUM") as scp,
            tc.tile_pool(name="pvp", bufs=1, space="PSUM") as pvp,
        ):
            # ---- constants ----
            wq = constp.tile([128, 4, 128], bf16, tag="wq")
            nc.sync.dma_start(
                out=wq, in_=wq_d.ap().rearrange("(c p) m -> p c m", p=128)
            )
            wk = constp.tile([128, 4, 128], bf16, tag="wk")
            nc.sync.dma_start(
                out=wk, in_=wk_d.ap().rearrange("(c p) m -> p c m", p=128)
            )
            wv = constp.tile([128, 4, 128], bf16, tag="wv")
            nc.sync.dma_start(
                out=wv, in_=wv_d.ap().rearrange("(c p) m -> p c m", p=128)
            )
            bq = constp.tile([128, 1], f32, tag="bq")
            nc.sync.dma_start(out=bq, in_=bq_d.ap())
            bk = constp.tile([128, 1], f32, tag="bk")
            nc.sync.dma_start(out=bk, in_=bk_d.ap())
            bv = constp.tile([128, 1], f32, tag="bv")
            nc.sync.dma_start(out=bv, in_=bv_d.ap())
            wo = constp.tile([128, 512], bf16, tag="wo")
            nc.sync.dma_start(out=wo, in_=wo_d.ap())
            ident = constp.tile([128, 64], f32, tag="ident")
            nc.sync.dma_start(out=ident, in_=id_d.ap())

            # ---- x^T load, split so projections can start early ----
            xT = bigp.tile([128, 4, S], bf16, tag="xT")
            xT_src = xT_d.ap().rearrange("(c p) s -> p c s", p=128)
            for j in range(4):
                for h in range(2):
                    nc.sync.dma_start(
                        out=xT[:, j, ts(h, 2048)], in_=xT_src[:, j, ts(h, 2048)]
                    )

            # ---- projections, one tile per 512-wide q/k slice ----
            # Emission order K(t) -> Q(t) -> V(t) so the attention stream
            # (which needs K t0 + Q t0/t1 first) starts as early as possible.
            QT2 = [
                bigp.tile([128, 512], bf16, tag=f"QT2_{t}", name="qt")
                for t in range(NT)
            ]
            KT2 = [
                bigp.tile([128, 512], bf16, tag=f"KT2_{t}", name="kt")
                for t in range(NT)
            ]
            VT2 = [
                bigp.tile([128, 512], f32, tag=f"VT2_{t}", name="vt")
                for t in range(NT)
            ]
            # V in natural layout per 128-k-chunk, ones in column 64
            V0 = [
                bigp.tile([128, 65], bf16, tag=f"V0_{ck}", name="v0")
                for ck in range(NCK)
            ]
            V1 = [
                bigp.tile([128, 65], bf16, tag=f"V1_{ck}", name="v1")
                for ck in range(NCK)
            ]

            _ptag = [0]

            def proj(t, w_sb, b_sb, dst):
                pps = pvp.tile([128, 512], f32, tag=f"pv{_ptag[0] % 4}", name="pps")
                _ptag[0] += 1
                for j in range(4):
                    nc.tensor.matmul(
                        pps,
                        w_sb[:, j, :],
                        xT[:, j, ts(t, 512)],
                        start=(j == 0),
                        stop=(j == 3),
                    )
                nc.vector.tensor_scalar_add(out=dst, in0=pps, scalar1=b_sb)

            def transposes(t):
                for i in range(4):
                    ck = 4 * t + i
                    for h, V in ((0, V0), (1, V1)):
                        vtp = pvp.tile(
                            [128, 64], f32, tag=f"pv{_ptag[0] % 4}", name="vtp"
                        )
                        _ptag[0] += 1
                        nc.tensor.transpose(
                            out=vtp,
                            in_=VT2[t][64 * h : 64 * h + 64, ts(i, 128)],
                            identity=ident[64 * h : 64 * h + 64, :],
                        )
                        nc.vector.tensor_copy(out=V[ck][:, 0:64], in_=vtp)
                        nc.vector.memset(V[ck][:, 64:65], 1.0)

            # K t0 / Q t0 / Q t1 first: they gate the first exp() call
            proj(0, wk, bk, KT2[0])
            proj(0, wq, bq, QT2[0])
            proj(1, wq, bq, QT2[1])
            proj(0, wv, bv, VT2[0])
            transposes(0)
            proj(1, wk, bk, KT2[1])
            proj(1, wv, bv, VT2[1])
            transposes(1)
            for t in range(2, NT):
                proj(t, wk, bk, KT2[t])
                proj(t, wq, bq, QT2[t])
                proj(t, wv, bv, VT2[t])
                transposes(t)

            # ---- attention ----
            attnT = [
                bigp.tile([128, 512], bf16, tag=f"attnT_{t}", name="at")
                for t in range(NT)
            ]

            def part_a(tp_i, pv):
                # pv-slot readers only: must be emitted before the next
                # tpair's first pv matmul reuses the slots.
                outs = []
                for par in range(2):
                    t = 2 * tp_i + par
                    den0 = stagep.tile([1, 512], f32, tag="den0", name="den0")
                    den1 = stagep.tile([1, 512], f32, tag="den1", name="den1")
                    nc.vector.tensor_copy(out=den0, in_=pv[0][par][64:65, :])
                    nc.vector.tensor_copy(out=den1, in_=pv[1][par][64:65, :])
                    nc.vector.tensor_copy(
                        out=attnT[t][0:64, :], in_=pv[0][par][0:64, :]
                    )
                    nc.vector.tensor_copy(
                        out=attnT[t][64:128, :], in_=pv[1][par][0:64, :]
                    )
                    outs.append((t, den0, den1))
                return outs

            def part_b(dens):
                # off the critical path: reciprocal + broadcast + normalize
                for t, den0, den1 in dens:
                    rc0 = stagep.tile([1, 512], f32, tag="rc0", name="rc0")
                    rc1 = stagep.tile([1, 512], f32, tag="rc1", name="rc1")
                    nc.vector.reciprocal_approx_fast(out=rc0, in_=den0)
                    nc.vector.reciprocal_approx_fast(out=rc1, in_=den1)
                    bct0 = stagep.tile([128, 512], f32, tag="bct0", name="bct0")
                    bct1 = stagep.tile([128, 512], f32, tag="bct1", name="bct1")
                    nc.gpsimd.partition_broadcast(bct0, rc0)
                    nc.gpsimd.partition_broadcast(bct1, rc1)
                    nc.vector.tensor_mul(
                        attnT[t][0:64, :], attnT[t][0:64, :], bct0[0:64, :]
                    )
                    nc.vector.tensor_mul(
                        attnT[t][64:128, :], attnT[t][64:128, :], bct1[64:128, :]
                    )

            pending = None
            for tp_i in range(NT // 2):
                pv = [
                    [
                        pvp.tile([65, 512], f32, tag=f"pv{2 * h + par}", name="pv")
                        for par in range(2)
                    ]
                    for h in range(2)
                ]

                def emit_pv(ck, exs, pv=pv):
                    for par in range(2):
                        nc.tensor.matmul(
                            pv[0][par],
                            V0[ck],
                            exs[par][:, 0:512],
                            start=(ck == 0),
                            stop=(ck == NCK - 1),
                        )
                        nc.tensor.matmul(
                            pv[1][par],
                            V1[ck],
                            exs[par][:, 512:1024],
                            start=(ck == 0),
                            stop=(ck == NCK - 1),
                        )

                prev = None
                for ck in range(NCK):
                    exs = []
                    for par in range(2):
                        t = 2 * tp_i + par
                        sc = scp.tile([128, 1024], f32, tag=f"sc{par}", name="sc")
                        nc.tensor.matmul(
                            sc[:, 0:512],
                            KT2[ck // 4][0:64, ts(ck % 4, 128)],
                            QT2[t][0:64, :],
                            start=True,
                            stop=True,
                        )
                        nc.tensor.matmul(
                            sc[:, 512:1024],
                            KT2[ck // 4][64:128, ts(ck % 4, 128)],
                            QT2[t][64:128, :],
                            start=True,
                            stop=True,
                        )
                        ex = expool.tile([128, 1024], bf16, tag=f"ex{par}", name="ex")
                        nc.scalar.activation(out=ex, in_=sc, func=Exp, scale=0.125)
                        exs.append(ex)
                    if prev is not None:
                        emit_pv(prev[0], prev[1])
                    prev = (ck, exs)
                    if ck == 2 and pending is not None:
                        # previous tpair's normalization, now well clear of
                        # the scalar-engine stream restart
                        part_b(pending)
                        pending = None
                emit_pv(prev[0], prev[1])
                pending = part_a(tp_i, pv)
            part_b(pending)

            # ---- output projection (all q-tiles at the end) ----
            for t in range(NT):
                for m in range(4):
                    ops = scp.tile([128, 1024], f32, tag=f"sc{m % 2}", name="ops")
                    nc.tensor.matmul(
                        ops[:, 0:512],
                        wo[:, ts(m, 128)],
                        attnT[t],
                        start=True,
                        stop=True,
                    )
                    ost = ostp.tile([128, 512], f32, tag=f"ostage{m % 2}", name="ost")
                    if m % 2 == 0:
                        nc.vector.tensor_copy(out=ost, in_=ops[:, 0:512])
                    else:
                        nc.scalar.copy(out=ost, in_=ops[:, 0:512])
                    nc.sync.dma_start(
                        out=outT_d.ap()[ts(m, 128), ts(t, 512)], in_=ost
                    )

    nc.compile()
    return nc


def _get_nc():
    if "nc" not in _CACHE:
        _CACHE["nc"] = _build_nc()
    return _CACHE["nc"]


def _bf16np():
    import ml_dtypes

    return ml_dtypes.bfloat16


def _make_in_maps(inputs):
    x = np.ascontiguousarray(np.asarray(inputs["x"], dtype=np.float32))
    Wq = np.asarray(inputs["Wq"], dtype=np.float32)
    Wk = np.asarray(inputs["Wk"], dtype=np.float32)
    Wv = np.asarray(inputs["Wv"], dtype=np.float32)
    Wo = np.asarray(inputs["Wo"], dtype=np.float32)
    bq = np.asarray(inputs["bq"], dtype=np.float32)
    bk = np.asarray(inputs["bk"], dtype=np.float32)
    bv = np.asarray(inputs["bv"], dtype=np.float32)

    bf = _bf16np()
    ident = np.concatenate([np.eye(64, dtype=np.float32)] * 2, axis=0)

    in_maps = []
    for c in range(N_CORES):
        b, p = c // 4, c % 4
        hs = slice(128 * p, 128 * (p + 1))
        in_maps.append(
            {
                "xT": np.ascontiguousarray(x[b].T).astype(bf),
                "wq2": np.ascontiguousarray(Wq[hs, :].T).astype(bf),
                "wk2": np.ascontiguousarray(Wk[hs, :].T).astype(bf),
                "wv2": np.ascontiguousarray(Wv[hs, :].T).astype(bf),
                "bq2": np.ascontiguousarray(bq[hs]).reshape(128, 1),
                "bk2": np.ascontiguousarray(bk[hs]).reshape(128, 1),
                "bv2": np.ascontiguousarray(bv[hs]).reshape(128, 1),
                "wo2": np.ascontiguousarray(Wo[:, hs].T).astype(bf),
                "ident": ident,
            }
        )
    return in_maps


def _gather(results, inputs):
    bo = np.asarray(inputs["bo"], dtype=np.float32)
    out = np.zeros((B, S, D_MODEL), np.float32)
    for c in range(N_CORES):
        out[c // 4] += results[c]["outT"].T
    out += bo[None, None, :]
    return out


def kernel(**inputs):
    from concourse.bass_utils import run_bass_kernel_spmd

    nc = _get_nc()
    in_maps = _make_in_maps(inputs)
    res = run_bass_kernel_spmd(nc, in_maps, list(range(N_CORES)))
    return _gather(res.results, inputs)


# revision 15
# speedup vs baseline: 1.5739x; 1.0157x over previous
# Multi-head attention (B=2, S=4096, D=512, H=8) on 8 trn2 NeuronCores.
#
# Sharding: core c -> batch b=c//4, head-pair p=c%4 (heads 2p, 2p+1).
# Each core computes its two heads' attention plus the partial output
# projection restricted to those heads' columns of Wo; the host sums the
# 4 partials per batch and adds bo. No cross-device communication.
#
# Device-side layout is fully "transposed": Q^T/K^T [head_dim, S] come
# straight out of the projection matmuls (weights stationary, x^T
# streaming), scores are computed as S^T[k, q] so the PV matmul needs no
# transposes, and a ones-column appended to V makes the PV accumulation
# produce softmax denominators for free. exp() runs on the scalar engine
# reading score PSUM directly (scale=1/8 folded in); softmax max-
# subtraction is skipped (scores are O(1) here, exp cannot overflow).
# Attention matmuls run in bf16 (softmax normalization + long averaging
# damps the rounding noise); x^T is shipped as bf16 which also halves
# the input DMA. The scalar engine (exp) is the bottleneck; the PV
# matmuls are software-pipelined one k-chunk behind the scores so the
# in-order PE never waits on the exp it just triggered, and the
# normalization/output-projection epilogues are kept off the scalar
# engine's critical path.

import numpy as np

D_MODEL = 512
NUM_HEADS = 8
D_K = 64
B, S = 2, 4096
N_CORES = 8

_CACHE = {}


def _build_nc():
    from concourse import bacc, mybir
    import concourse.tile as tile
    from concourse.bass import ts

    f32 = mybir.dt.float32
    bf16 = mybir.dt.bfloat16
    Exp = mybir.ActivationFunctionType.Exp

    nc = bacc.Bacc("TRN2", target_bir_lowering=False, debug=False)

    xT_d = nc.dram_tensor("xT", [512, S], bf16, kind="ExternalInput")
    wq_d = nc.dram_tensor("wq2", [512, 128], bf16, kind="ExternalInput")
    wk_d = nc.dram_tensor("wk2", [512, 128], bf16, kind="ExternalInput")
    wv_d = nc.dram_tensor("wv2", [512, 128], bf16, kind="ExternalInput")
    bq_d = nc.dram_tensor("bq2", [128, 1], f32, kind="ExternalInput")
    bk_d = nc.dram_tensor("bk2", [128, 1], f32, kind="ExternalInput")
    bv_d = nc.dram_tensor("bv2", [128, 1], f32, kind="ExternalInput")
    wo_d = nc.dram_tensor("wo2", [128, 512], bf16, kind="ExternalInput")
    outT_d = nc.dram_tensor("outT", [512, S], f32, kind="ExternalOutput")

    NT = S // 512  # 8 q-tiles of 512
    NCK = S // 128  # 32 k-chunks of 128

    with tile.TileContext(nc) as tc:
        with (
            tc.tile_pool(name="const", bufs=1) as constp,
            tc.tile_pool(name="big", bufs=1) as bigp,
            tc.tile_pool(name="expool", bufs=2) as expool,
            tc.tile_pool(name="stage", bufs=2) as stagep,
            tc.tile_pool(name="ost", bufs=3) as ostp,
            tc.tile_pool(name="scp", bufs=1, space="PSUM") as scp,
            tc.tile_pool(name="pvp", bufs=1, space="PSUM") as pvp,
        ):
            # ---- constants ----
            wq = constp.tile([128, 4, 128], bf16, tag="wq")
            nc.sync.dma_start(
                out=wq, in_=wq_d.ap().rearrange("(c p) m -> p c m", p=128)
            )
            wk = constp.tile([128, 4, 128], bf16, tag="wk")
            nc.sync.dma_start(
                out=wk, in_=wk_d.ap().rearrange("(c p) m -> p c m", p=128)
            )
            wv = constp.tile([128, 4, 128], bf16, tag="wv")
            nc.sync.dma_start(
                out=wv, in_=wv_d.ap().rearrange("(c p) m -> p c m", p=128)
            )
            bq = constp.tile([128, 1], f32, tag="bq")
            nc.sync.dma_start(out=bq, in_=bq_d.ap())
            bk = constp.tile([128, 1], f32, tag="bk")
            nc.sync.dma_start(out=bk, in_=bk_d.ap())
            bv = constp.tile([128, 1], f32, tag="bv")
            nc.sync.dma_start(out=bv, in_=bv_d.ap())
            wo = constp.tile([128, 512], bf16, tag="wo")
            nc.sync.dma_start(out=wo, in_=wo_d.ap())

            # ---- x^T load, split so projections can start early ----
            xT = bigp.tile([128, 4, S], bf16, tag="xT")
            xT_src = xT_d.ap().rearrange("(c p) s -> p c s", p=128)
            for j in range(4):
                for h in range(2):
                    nc.sync.dma_start(
                        out=xT[:, j, ts(h, 2048)], in_=xT_src[:, j, ts(h, 2048)]
                    )

            # ---- projections, one tile per 512-wide q/k slice ----
            # Emission order K(t) -> Q(t) -> V(t) so the attention stream
            # (which needs K t0 + Q t0/t1 first) starts as early as possible.
            QT2 = [
                bigp.tile([128, 512], bf16, tag=f"QT2_{t}", name="qt")
                for t in range(NT)
            ]
            KT2 = [
                bigp.tile([128, 512], bf16, tag=f"KT2_{t}", name="kt")
                for t in range(NT)
            ]
            VT2 = [
                bigp.tile([128, 512], bf16, tag=f"VT2_{t}", name="vt")
                for t in range(NT)
            ]
            # V in natural layout per 128-k-chunk, ones in column 64
            V0 = [
                bigp.tile([128, 65], bf16, tag=f"V0_{ck}", name="v0")
                for ck in range(NCK)
            ]
            V1 = [
                bigp.tile([128, 65], bf16, tag=f"V1_{ck}", name="v1")
                for ck in range(NCK)
            ]

            _ptag = [0]

            def proj(t, w_sb, b_sb, dst):
                pps = pvp.tile([128, 512], f32, tag=f"pv{_ptag[0] % 4}", name="pps")
                _ptag[0] += 1
                for j in range(4):
                    nc.tensor.matmul(
                        pps,
                        w_sb[:, j, :],
                        xT[:, j, ts(t, 512)],
                        start=(j == 0),
                        stop=(j == 3),
                    )
                nc.vector.tensor_scalar_add(out=dst, in0=pps, scalar1=b_sb)

            def transposes(t):
                for i in range(4):
                    ck = 4 * t + i
                    for h, V in ((0, V0), (1, V1)):
                        nc.sync.dma_start(
                            out=V[ck][:, 0:64],
                            in_=VT2[t][64 * h : 64 * h + 64, ts(i, 128)],
                            transpose=True,
                        )
                        nc.vector.memset(V[ck][:, 64:65], 1.0)

            # K t0 / Q t0 / Q t1 first: they gate the first exp() call
            proj(0, wk, bk, KT2[0])
            proj(0, wq, bq, QT2[0])
            proj(1, wq, bq, QT2[1])
            proj(0, wv, bv, VT2[0])
            transposes(0)
            proj(1, wk, bk, KT2[1])
            proj(1, wv, bv, VT2[1])
            transposes(1)
            for t in range(2, NT):
                proj(t, wk, bk, KT2[t])
                proj(t, wq, bq, QT2[t])
                proj(t, wv, bv, VT2[t])
                transposes(t)

            # ---- attention ----
            attnT = [
                bigp.tile([128, 512], bf16, tag=f"attnT_{t}", name="at")
                for t in range(NT)
            ]

            def part_a(tp_i, pv):
                # pv-slot readers only: must be emitted before the next
                # tpair's first pv matmul reuses the slots.
                outs = []
                for par in range(2):
                    t = 2 * tp_i + par
                    den0 = stagep.tile([1, 512], f32, tag="den0", name="den0")
                    den1 = stagep.tile([1, 512], f32, tag="den1", name="den1")
                    nc.vector.tensor_copy(out=den0, in_=pv[0][par][64:65, :])
                    nc.vector.tensor_copy(out=den1, in_=pv[1][par][64:65, :])
                    nc.vector.tensor_copy(
                        out=attnT[t][0:64, :], in_=pv[0][par][0:64, :]
                    )
                    nc.vector.tensor_copy(
                        out=attnT[t][64:128, :], in_=pv[1][par][0:64, :]
                    )
                    outs.append((t, den0, den1))
                return outs

            def part_b(dens):
                # off the critical path: reciprocal + broadcast + normalize
                for t, den0, den1 in dens:
                    rc0 = stagep.tile([1, 512], f32, tag="rc0", name="rc0")
                    rc1 = stagep.tile([1, 512], f32, tag="rc1", name="rc1")
                    nc.vector.reciprocal_approx_fast(out=rc0, in_=den0)
                    nc.vector.reciprocal_approx_fast(out=rc1, in_=den1)
                    bct0 = stagep.tile([128, 512], f32, tag="bct0", name="bct0")
                    bct1 = stagep.tile([128, 512], f32, tag="bct1", name="bct1")
                    nc.gpsimd.partition_broadcast(bct0, rc0)
                    nc.gpsimd.partition_broadcast(bct1, rc1)
                    nc.vector.tensor_mul(
                        attnT[t][0:64, :], attnT[t][0:64, :], bct0[0:64, :]
                    )
                    nc.vector.tensor_mul(
                        attnT[t][64:128, :], attnT[t][64:128, :], bct1[64:128, :]
                    )

            pending = None
            for tp_i in range(NT // 2):
                pv = [
                    [
                        pvp.tile([65, 512], f32, tag=f"pv{2 * h + par}", name="pv")
                        for par in range(2)
                    ]
                    for h in range(2)
                ]

                def emit_pv(ck, exs, pv=pv):
                    for par in range(2):
                        nc.tensor.matmul(
                            pv[0][par],
                            V0[ck],
                            exs[par][:, 0:512],
                            start=(ck == 0),
                            stop=(ck == NCK - 1),
                        )
                        nc.tensor.matmul(
                            pv[1][par],
                            V1[ck],
                            exs[par][:, 512:1024],
                            start=(ck == 0),
                            stop=(ck == NCK - 1),
                        )

                prev = None
                for ck in range(NCK):
                    exs = []
                    for par in range(2):
                        t = 2 * tp_i + par
                        sc = scp.tile([128, 1024], f32, tag=f"sc{par}", name="sc")
                        nc.tensor.matmul(
                            sc[:, 0:512],
                            KT2[ck // 4][0:64, ts(ck % 4, 128)],
                            QT2[t][0:64, :],
                            start=True,
                            stop=True,
                        )
                        nc.tensor.matmul(
                            sc[:, 512:1024],
                            KT2[ck // 4][64:128, ts(ck % 4, 128)],
                            QT2[t][64:128, :],
                            start=True,
                            stop=True,
                        )
                        ex = expool.tile([128, 1024], bf16, tag=f"ex{par}", name="ex")
                        nc.scalar.activation(out=ex, in_=sc, func=Exp, scale=0.125)
                        exs.append(ex)
                    if prev is not None:
                        emit_pv(prev[0], prev[1])
                    prev = (ck, exs)
                    if ck == 2 and pending is not None:
                        # previous tpair's normalization, now well clear of
                        # the scalar-engine stream restart
                        part_b(pending)
                        pending = None
                emit_pv(prev[0], prev[1])
                pending = part_a(tp_i, pv)
            part_b(pending)

            # ---- output projection (all q-tiles at the end) ----
            for t in range(NT):
                for m in range(4):
                    ops = scp.tile([128, 1024], f32, tag=f"sc{m % 2}", name="ops")
                    nc.tensor.matmul(
                        ops[:, 0:512],
                        wo[:, ts(m, 128)],
                        attnT[t],
                        start=True,
                        stop=True,
                    )
                    ost = ostp.tile([128, 512], f32, tag=f"ostage{m % 2}", name="ost")
                    if m % 2 == 0:
                        nc.vector.tensor_copy(out=ost, in_=ops[:, 0:512])
                    else:
                        nc.scalar.copy(out=ost, in_=ops[:, 0:512])
                    nc.sync.dma_start(
                        out=outT_d.ap()[ts(m, 128), ts(t, 512)], in_=ost
                    )

    nc.compile()
    return nc


def _get_nc():
    if "nc" not in _CACHE:
        _CACHE["nc"] = _build_nc()
    return _CACHE["nc"]


def _bf16np():
    import ml_dtypes

    return ml_dtypes.bfloat16


def _make_in_maps(inputs):
    x = np.ascontiguousarray(np.asarray(inputs["x"], dtype=np.float32))
    Wq = np.asarray(inputs["Wq"], dtype=np.float32)
    Wk = np.asarray(inputs["Wk"], dtype=np.float32)
    Wv = np.asarray(inputs["Wv"], dtype=np.float32)
    Wo = np.asarray(inputs["Wo"], dtype=np.float32)
    bq = np.asarray(inputs["bq"], dtype=np.float32)
    bk = np.asarray(inputs["bk"], dtype=np.float32)
    bv = np.asarray(inputs["bv"], dtype=np.float32)

    bf = _bf16np()

    in_maps = []
    for c in range(N_CORES):
        b, p = c // 4, c % 4
        hs = slice(128 * p, 128 * (p + 1))
        in_maps.append(
            {
                "xT": np.ascontiguousarray(x[b].T).astype(bf),
                "wq2": np.ascontiguousarray(Wq[hs, :].T).astype(bf),
                "wk2": np.ascontiguousarray(Wk[hs, :].T).astype(bf),
                "wv2": np.ascontiguousarray(Wv[hs, :].T).astype(bf),
                "bq2": np.ascontiguousarray(bq[hs]).reshape(128, 1),
                "bk2": np.ascontiguousarray(bk[hs]).reshape(128, 1),
                "bv2": np.ascontiguousarray(bv[hs]).reshape(128, 1),
                "wo2": np.ascontiguousarray(Wo[:, hs].T).astype(bf),
            }
        )
    return in_maps


def _gather(results, inputs):
    bo = np.asarray(inputs["bo"], dtype=np.float32)
    out = np.zeros((B, S, D_MODEL), np.float32)
    for c in range(N_CORES):
        out[c // 4] += results[c]["outT"].T
    out += bo[None, None, :]
    return out


def kernel(**inputs):
    from concourse.bass_utils import run_bass_kernel_spmd

    nc = _get_nc()
    in_maps = _make_in_maps(inputs)
    res = run_bass_kernel_spmd(nc, in_maps, list(range(N_CORES)))
    return _gather(res.results, inputs)


# revision 16
# speedup vs baseline: 1.6131x; 1.0249x over previous
# Multi-head attention (B=2, S=4096, D=512, H=8) on 8 trn2 NeuronCores.
#
# Sharding: core c -> batch b=c//4, head-pair p=c%4 (heads 2p, 2p+1).
# Each core computes its two heads' attention plus the partial output
# projection restricted to those heads' columns of Wo; the host sums the
# 4 partials per batch and adds bo. No cross-device communication.
#
# Device-side layout is fully "transposed": Q^T/K^T [head_dim, S] come
# straight out of the projection matmuls (weights stationary, x^T
# streaming), scores are computed as S^T[k, q] so the PV matmul needs no
# transposes, and a ones-column appended to V makes the PV accumulation
# produce softmax denominators for free. exp() runs on the scalar engine
# reading score PSUM directly (scale=1/8 folded in); softmax max-
# subtraction is skipped (scores are O(1) here, exp cannot overflow).
# Attention matmuls run in bf16 (softmax normalization + long averaging
# damps the rounding noise); x^T is shipped as bf16 which also halves
# the input DMA. The scalar engine (exp) is the bottleneck; the PV
# matmuls are software-pipelined one k-chunk behind the scores so the
# in-order PE never waits on the exp it just triggered, and the
# normalization/output-projection epilogues are kept off the scalar
# engine's critical path.

import numpy as np

D_MODEL = 512
NUM_HEADS = 8
D_K = 64
B, S = 2, 4096
N_CORES = 8

_CACHE = {}


def _build_nc():
    from concourse import bacc, mybir
    import concourse.tile as tile
    from concourse.bass import ts

    f32 = mybir.dt.float32
    bf16 = mybir.dt.bfloat16
    Exp = mybir.ActivationFunctionType.Exp

    nc = bacc.Bacc("TRN2", target_bir_lowering=False, debug=False)

    xT_d = nc.dram_tensor("xT", [512, S], bf16, kind="ExternalInput")
    wq_d = nc.dram_tensor("wq2", [512, 128], bf16, kind="ExternalInput")
    wk_d = nc.dram_tensor("wk2", [512, 128], bf16, kind="ExternalInput")
    wv_d = nc.dram_tensor("wv2", [512, 128], bf16, kind="ExternalInput")
    bq_d = nc.dram_tensor("bq2", [128, 1], f32, kind="ExternalInput")
    bk_d = nc.dram_tensor("bk2", [128, 1], f32, kind="ExternalInput")
    bv_d = nc.dram_tensor("bv2", [128, 1], f32, kind="ExternalInput")
    wo_d = nc.dram_tensor("wo2", [128, 512], bf16, kind="ExternalInput")
    outT_d = nc.dram_tensor("outT", [512, S], f32, kind="ExternalOutput")

    NT = S // 512  # 8 q-tiles of 512
    NCK = S // 128  # 32 k-chunks of 128

    with tile.TileContext(nc) as tc:
        with (
            tc.tile_pool(name="const", bufs=1) as constp,
            tc.tile_pool(name="big", bufs=1) as bigp,
            tc.tile_pool(name="expool", bufs=2) as expool,
            tc.tile_pool(name="stage", bufs=2) as stagep,
            tc.tile_pool(name="ost", bufs=3) as ostp,
            tc.tile_pool(name="scp", bufs=1, space="PSUM") as scp,
            tc.tile_pool(name="pvp", bufs=1, space="PSUM") as pvp,
        ):
            # ---- constants ----
            wq = constp.tile([128, 4, 128], bf16, tag="wq")
            nc.sync.dma_start(
                out=wq, in_=wq_d.ap().rearrange("(c p) m -> p c m", p=128)
            )
            wk = constp.tile([128, 4, 128], bf16, tag="wk")
            nc.sync.dma_start(
                out=wk, in_=wk_d.ap().rearrange("(c p) m -> p c m", p=128)
            )
            wv = constp.tile([128, 4, 128], bf16, tag="wv")
            nc.sync.dma_start(
                out=wv, in_=wv_d.ap().rearrange("(c p) m -> p c m", p=128)
            )
            bq = constp.tile([128, 1], f32, tag="bq")
            nc.sync.dma_start(out=bq, in_=bq_d.ap())
            bk = constp.tile([128, 1], f32, tag="bk")
            nc.sync.dma_start(out=bk, in_=bk_d.ap())
            bv = constp.tile([128, 1], f32, tag="bv")
            nc.sync.dma_start(out=bv, in_=bv_d.ap())
            wo = constp.tile([128, 512], bf16, tag="wo")
            nc.sync.dma_start(out=wo, in_=wo_d.ap())

            # ---- x^T load: 8 independent tiles so each projection can
            # start as soon as its own chunks have landed ----
            xTt = [
                [bigp.tile([128, 2048], bf16, tag=f"xT_{j}_{h}", name="xc") for h in range(2)]
                for j in range(4)
            ]
            xT_src = xT_d.ap().rearrange("(c p) s -> p c s", p=128)
            for h in range(2):
                for j in range(4):
                    nc.sync.dma_start(
                        out=xTt[j][h], in_=xT_src[:, j, ts(h, 2048)]
                    )

            # warm the PE (HAM un-throttle) while the x^T DMA is in flight
            junk = bigp.tile([128, 512], bf16, tag="junk")
            nc.vector.memset(junk, 0.0)
            for w in range(20):
                jp = scp.tile([128, 1024], f32, tag=f"sc{w % 2}", name="jp")
                nc.tensor.matmul(
                    jp[:, 0:512], junk[:, 0:128], junk, start=True, stop=True
                )

            # ---- projections, one tile per 512-wide q/k slice ----
            # Emission order K(t) -> Q(t) -> V(t) so the attention stream
            # (which needs K t0 + Q t0/t1 first) starts as early as possible.
            QT2 = [
                bigp.tile([128, 512], bf16, tag=f"QT2_{t}", name="qt")
                for t in range(NT)
            ]
            KT2 = [
                bigp.tile([128, 512], bf16, tag=f"KT2_{t}", name="kt")
                for t in range(NT)
            ]
            VT2 = [
                bigp.tile([128, 512], bf16, tag=f"VT2_{t}", name="vt")
                for t in range(NT)
            ]
            # V in natural layout per 128-k-chunk, ones in column 64
            V0 = [
                bigp.tile([128, 65], bf16, tag=f"V0_{ck}", name="v0")
                for ck in range(NCK)
            ]
            V1 = [
                bigp.tile([128, 65], bf16, tag=f"V1_{ck}", name="v1")
                for ck in range(NCK)
            ]

            _ptag = [0]

            def proj(t, w_sb, b_sb, dst):
                pps = pvp.tile([128, 512], f32, tag=f"pv{_ptag[0] % 4}", name="pps")
                _ptag[0] += 1
                for j in range(4):
                    nc.tensor.matmul(
                        pps,
                        w_sb[:, j, :],
                        xTt[j][t // 4][:, ts(t % 4, 512)],
                        start=(j == 0),
                        stop=(j == 3),
                    )
                nc.vector.tensor_scalar_add(out=dst, in0=pps, scalar1=b_sb)

            def transposes(t):
                for i in range(4):
                    ck = 4 * t + i
                    for h, V in ((0, V0), (1, V1)):
                        nc.sync.dma_start(
                            out=V[ck][:, 0:64],
                            in_=VT2[t][64 * h : 64 * h + 64, ts(i, 128)],
                            transpose=True,
                        )
                        nc.vector.memset(V[ck][:, 64:65], 1.0)

            # K t0 / Q t0 / Q t1 first: they gate the first exp() call
            proj(0, wk, bk, KT2[0])
            proj(0, wq, bq, QT2[0])
            proj(1, wq, bq, QT2[1])
            proj(0, wv, bv, VT2[0])
            transposes(0)
            proj(1, wk, bk, KT2[1])
            proj(1, wv, bv, VT2[1])
            transposes(1)
            for t in range(2, NT):
                proj(t, wk, bk, KT2[t])
                proj(t, wq, bq, QT2[t])
                proj(t, wv, bv, VT2[t])
                transposes(t)

            # ---- attention ----
            attnT = [
                bigp.tile([128, 512], bf16, tag=f"attnT_{t}", name="at")
                for t in range(NT)
            ]

            def part_a(tp_i, pv):
                # pv-slot readers only: must be emitted before the next
                # tpair's first pv matmul reuses the slots.
                outs = []
                for par in range(2):
                    t = 2 * tp_i + par
                    den0 = stagep.tile([1, 512], f32, tag="den0", name="den0")
                    den1 = stagep.tile([1, 512], f32, tag="den1", name="den1")
                    nc.vector.tensor_copy(out=den0, in_=pv[0][par][64:65, :])
                    nc.vector.tensor_copy(out=den1, in_=pv[1][par][64:65, :])
                    nc.vector.tensor_copy(
                        out=attnT[t][0:64, :], in_=pv[0][par][0:64, :]
                    )
                    nc.vector.tensor_copy(
                        out=attnT[t][64:128, :], in_=pv[1][par][0:64, :]
                    )
                    outs.append((t, den0, den1))
                return outs

            def part_b(dens):
                # off the critical path: reciprocal + broadcast + normalize
                for t, den0, den1 in dens:
                    rc0 = stagep.tile([1, 512], f32, tag="rc0", name="rc0")
                    rc1 = stagep.tile([1, 512], f32, tag="rc1", name="rc1")
                    nc.vector.reciprocal_approx_fast(out=rc0, in_=den0)
                    nc.vector.reciprocal_approx_fast(out=rc1, in_=den1)
                    bct0 = stagep.tile([128, 512], f32, tag="bct0", name="bct0")
                    bct1 = stagep.tile([128, 512], f32, tag="bct1", name="bct1")
                    nc.gpsimd.partition_broadcast(bct0, rc0)
                    nc.gpsimd.partition_broadcast(bct1, rc1)
                    nc.vector.tensor_mul(
                        attnT[t][0:64, :], attnT[t][0:64, :], bct0[0:64, :]
                    )
                    nc.vector.tensor_mul(
                        attnT[t][64:128, :], attnT[t][64:128, :], bct1[64:128, :]
                    )

            pending = None
            for tp_i in range(NT // 2):
                pv = [
                    [
                        pvp.tile([65, 512], f32, tag=f"pv{2 * h + par}", name="pv")
                        for par in range(2)
                    ]
                    for h in range(2)
                ]

                def emit_pv(ck, exs, pv=pv):
                    for par in range(2):
                        nc.tensor.matmul(
                            pv[0][par],
                            V0[ck],
                            exs[par][:, 0:512],
                            start=(ck == 0),
                            stop=(ck == NCK - 1),
                        )
                        nc.tensor.matmul(
                            pv[1][par],
                            V1[ck],
                            exs[par][:, 512:1024],
                            start=(ck == 0),
                            stop=(ck == NCK - 1),
                        )

                prev = None
                for ck in range(NCK):
                    exs = []
                    for par in range(2):
                        t = 2 * tp_i + par
                        sc = scp.tile([128, 1024], f32, tag=f"sc{par}", name="sc")
                        nc.tensor.matmul(
                            sc[:, 0:512],
                            KT2[ck // 4][0:64, ts(ck % 4, 128)],
                            QT2[t][0:64, :],
                            start=True,
                            stop=True,
                        )
                        nc.tensor.matmul(
                            sc[:, 512:1024],
                            KT2[ck // 4][64:128, ts(ck % 4, 128)],
                            QT2[t][64:128, :],
                            start=True,
                            stop=True,
                        )
                        ex = expool.tile([128, 1024], bf16, tag=f"ex{par}", name="ex")
                        nc.scalar.activation(out=ex, in_=sc, func=Exp, scale=0.125)
                        exs.append(ex)
                    if prev is not None:
                        emit_pv(prev[0], prev[1])
                    prev = (ck, exs)
                    if ck == 2 and pending is not None:
                        # previous tpair's normalization, now well clear of
                        # the scalar-engine stream restart
                        part_b(pending)
                        pending = None
                emit_pv(prev[0], prev[1])
                pending = part_a(tp_i, pv)
            part_b(pending)

            # ---- output projection (all q-tiles at the end) ----
            for t in range(NT):
                for m in range(4):
                    ops = scp.tile([128, 1024], f32, tag=f"sc{m % 2}", name="ops")
                    nc.tensor.matmul(
                        ops[:, 0:512],
                        wo[:, ts(m, 128)],
                        attnT[t],
                        start=True,
                        stop=True,
                    )
                    ost = ostp.tile([128, 512], f32, tag=f"ostage{m % 2}", name="ost")
                    if m % 2 == 0:
                        nc.vector.tensor_copy(out=ost, in_=ops[:, 0:512])
                    else:
                        nc.scalar.copy(out=ost, in_=ops[:, 0:512])
                    nc.sync.dma_start(
                        out=outT_d.ap()[ts(m, 128), ts(t, 512)], in_=ost
                    )

    nc.compile()
    return nc


def _get_nc():
    if "nc" not in _CACHE:
        _CACHE["nc"] = _build_nc()
    return _CACHE["nc"]


def _bf16np():
    import ml_dtypes

    return ml_dtypes.bfloat16


def _make_in_maps(inputs):
    x = np.ascontiguousarray(np.asarray(inputs["x"], dtype=np.float32))
    Wq = np.asarray(inputs["Wq"], dtype=np.float32)
    Wk = np.asarray(inputs["Wk"], dtype=np.float32)
    Wv = np.asarray(inputs["Wv"], dtype=np.float32)
    Wo = np.asarray(inputs["Wo"], dtype=np.float32)
    bq = np.asarray(inputs["bq"], dtype=np.float32)
    bk = np.asarray(inputs["bk"], dtype=np.float32)
    bv = np.asarray(inputs["bv"], dtype=np.float32)

    bf = _bf16np()

    in_maps = []
    for c in range(N_CORES):
        b, p = c // 4, c % 4
        hs = slice(128 * p, 128 * (p + 1))
        in_maps.append(
            {
                "xT": np.ascontiguousarray(x[b].T).astype(bf),
                "wq2": np.ascontiguousarray(Wq[hs, :].T).astype(bf),
                "wk2": np.ascontiguousarray(Wk[hs, :].T).astype(bf),
                "wv2": np.ascontiguousarray(Wv[hs, :].T).astype(bf),
                "bq2": np.ascontiguousarray(bq[hs]).reshape(128, 1),
                "bk2": np.ascontiguousarray(bk[hs]).reshape(128, 1),
                "bv2": np.ascontiguousarray(bv[hs]).reshape(128, 1),
                "wo2": np.ascontiguousarray(Wo[:, hs].T).astype(bf),
            }
        )
    return in_maps


def _gather(results, inputs):
    bo = np.asarray(inputs["bo"], dtype=np.float32)
    out = np.zeros((B, S, D_MODEL), np.float32)
    for c in range(N_CORES):
        out[c // 4] += results[c]["outT"].T
    out += bo[None, None, :]
    return out


def kernel(**inputs):
    from concourse.bass_utils import run_bass_kernel_spmd

    nc = _get_nc()
    in_maps = _make_in_maps(inputs)
    res = run_bass_kernel_spmd(nc, in_maps, list(range(N_CORES)))
    return _gather(res.results, inputs)
